# revision 41
# baseline (speedup 1.0000x reference)
"""CompressAttn Trainium2 Bass kernel (v9).

Problem: compressed-block attention.
  B=2, N=4096, QH=32, KH=2, D=VD=128, KSZ=32, STRIDE=16, M=255 blocks.
  kc[b,m,h,:] = sum_i w_k[i] * (k[b,16m+i,h,:] + pe_k[i,:])   (same for v)
  out = softmax(q @ kc^T * D^-0.5, causal-banded mask) @ vc, zero for n < 31.

Sharding: 8 cores = (batch b in {0,1}) x (query-head quarter hq in {0..3}).
Each core handles 8 query heads that share a single KV head (g = hq//2), so
K/V compression is done once per core.  No collectives needed; host gathers.

Device structure:
  - The causal staircase mask is ADDED into the QK psum by the tensor engine
    (selection-identity stationary x NEG-mask moving), so Scalar runs exactly
    one exp per (head, 512-query block) and Vector runs one reciprocal + one
    broadcast normalization multiply per block.
  - Mask variants and the identity matrices are generated on-device with
    affine_select (saves >1MB of startup DMA); k/v/q loads are split across
    the SP / Activation / Pool(SWDGE) DMA queues (~100GB/s each).
  - Head 0 is special-cased: K compression -> QK/exp of blocks 0-3 -> V
    compression (psum borrowed from the PV pool) -> PV backlog, so attention
    starts ~13us in instead of ~26us.
  - Per-block emission is software-pipelined (QK(b+1) before PV(b)) so the
    PE never waits for exp; outputs are written bf16 [128 part, 32 tile, vd]
    with two DMAs per head and the host untangles the tiling.
"""

import ml_dtypes
import numpy as np

import concourse.bacc as bacc
import concourse.mybir as mybir
import concourse.tile as tile
from concourse.bass_utils import run_bass_kernel_spmd

# Problem geometry (hardcoded per contest rules).
B, N, QH, KH, D, VD = 2, 4096, 32, 2, 128, 128
KSZ, STRIDE = 32, 16
M = (N - KSZ) // STRIDE + 1          # 255 compressed blocks (m = 0..254)
HPC = QH // 4                         # 8 query heads per core
NBLK = N // 512                       # 8 query blocks of 512
SM = float(D) ** -0.5
NEGM = -16384.0                       # mask add; exp(SM*(-16384+s)) == 0

F32 = mybir.dt.float32
BF16 = mybir.dt.bfloat16

KTA_E = 17 * 128                      # ktile split: chunks 0-16 / 17-31
VTA_E = 16 * 128                      # vtile split: chunks 0-15 / 16-31


def build_program():
    nc = bacc.Bacc("TRN2", target_bir_lowering=False, debug=False)

    qT_d = nc.dram_tensor("qT", [HPC, D, N], BF16, kind="ExternalInput")
    # k/v pre-arranged on host to the SBUF tile layout [r, c, d] so loads
    # are one fully-contiguous descriptor per partition
    k_d = nc.dram_tensor("kk", [128, 32 * D], BF16, kind="ExternalInput")
    v_d = nc.dram_tensor("vv", [128, 32 * D], BF16, kind="ExternalInput")
    w01k_d = nc.dram_tensor("w01k", [128, 16], BF16, kind="ExternalInput")
    w01v_d = nc.dram_tensor("w01v", [128, 16], BF16, kind="ExternalInput")
    bk_d = nc.dram_tensor("biask", [128, 1], F32, kind="ExternalInput")
    bv_d = nc.dram_tensor("biasv", [128, 1], F32, kind="ExternalInput")
    ones_d = nc.dram_tensor("ones1", [128, 2], BF16, kind="ExternalInput")
    o_d = nc.dram_tensor("o", [HPC, 128, N // 128, VD], BF16,
                         kind="ExternalOutput")

    with tile.TileContext(nc) as tc:
        with tc.tile_pool(name="consts", bufs=1) as cp:
            w01k = cp.tile([128, 16], BF16)
            w01v = cp.tile([128, 16], BF16)
            biask = cp.tile([128, 1], F32)
            biasv = cp.tile([128, 1], F32)
            maskv = cp.tile([128, 8 * 512], BF16)
            mvf = cp.tile([128, 512], F32)
            zf = cp.tile([128, 512], F32)
            ident = cp.tile([128, 128], F32)
            identb = cp.tile([128, 128], BF16)
            tmpb = cp.tile([128, 128], BF16)
            tmpb2 = cp.tile([128, 128], BF16)
            tmpf = cp.tile([128, 128], F32)
            tmpf2 = cp.tile([128, 128], F32)
            ktA1 = cp.tile([128, 9 * 128], BF16)
            ktA2 = cp.tile([128, 8 * 128], BF16)
            ktB = cp.tile([128, 32 * 128 - KTA_E], BF16)
            vtA = cp.tile([128, VTA_E], BF16)
            vtB = cp.tile([128, 32 * 128 - VTA_E], BF16)
            kcT = cp.tile([128, 256], BF16)       # [d, m] (col 255 zero pad)
            vcT = cp.tile([128, 256], F32)        # [d, t] staging
            vca0 = cp.tile([128, 130], BF16)      # [m 0:128,   vc|1|0]
            vca1 = cp.tile([128, 130], BF16)      # [m 128:255, vc|1|0]
            q0a = cp.tile([128, 2048], BF16)      # head-0 q, blocks 0-3
            q0b = cp.tile([128, 2048], BF16)      # head-0 q, blocks 4-7

            # --- DMA schedule: 3 queues in parallel ---
            nc.sync.dma_start(w01k[:, :], w01k_d.ap())
            nc.sync.dma_start(w01v[:, :], w01v_d.ap())
            nc.sync.dma_start(ktA1[:, :], k_d.ap()[:, 0 : 9 * 128])
            nc.sync.dma_start(q0a[:, :], qT_d.ap()[0, :, 0:2048])
            nc.sync.dma_start(ktB[:, :], k_d.ap()[:, KTA_E:])
            nc.scalar.dma_start(biask[:, :], bk_d.ap())
            nc.scalar.dma_start(biasv[:, :], bv_d.ap())
            nc.scalar.dma_start(ktA2[:, :], k_d.ap()[:, 9 * 128 : KTA_E])
            nc.scalar.dma_start(q0b[:, :], qT_d.ap()[0, :, 2048:4096])
            nc.gpsimd.dma_start(vtA[:, :], v_d.ap()[:, 0:VTA_E])
            nc.gpsimd.dma_start(vtB[:, :], v_d.ap()[:, VTA_E:])
            nc.gpsimd.dma_start(vca0[:, 128:130], ones_d.ap())
            nc.gpsimd.dma_start(vca1[:, 128:130], ones_d.ap())

            # --- on-device constants (Pool affine_select + DVE copies,
            # overlapping the DMA waits); identities first (every mask
            # matmul needs identb), then mask variants in usage order
            nc.vector.memset(tmpb[:, :], 1.0)
            nc.gpsimd.affine_select(
                tmpb2[:, :], tmpb[:, :], pattern=[[1, 128]],
                compare_op=mybir.AluOpType.is_ge, fill=0.0,
                base=0, channel_multiplier=-1,
            )
            nc.gpsimd.affine_select(
                identb[:, :], tmpb2[:, :], pattern=[[-1, 128]],
                compare_op=mybir.AluOpType.is_ge, fill=0.0,
                base=0, channel_multiplier=1,
            )
            nc.vector.memset(tmpf[:, :], 1.0)
            nc.gpsimd.affine_select(
                tmpf2[:, :], tmpf[:, :], pattern=[[1, 128]],
                compare_op=mybir.AluOpType.is_ge, fill=0.0,
                base=0, channel_multiplier=-1,
            )
            nc.gpsimd.affine_select(
                ident[:, :], tmpf2[:, :], pattern=[[-1, 128]],
                compare_op=mybir.AluOpType.is_ge, fill=0.0,
                base=0, channel_multiplier=1,
            )
            # mask variant v, row p, col j: NEGM iff j < 16*p + 31 - 512*v
            nc.vector.memset(zf[:, :], 0.0)
            for v in range(8):
                nc.gpsimd.affine_select(
                    mvf[:, :], zf[:, :], pattern=[[1, 512]],
                    compare_op=mybir.AluOpType.is_ge, fill=NEGM,
                    base=512 * v - 31, channel_multiplier=-16,
                )
                nc.vector.tensor_copy(maskv[:, 512 * v : 512 * v + 512],
                                      mvf[:, :])

            # ---- attention (+ compression woven into head 0) ----
            with (
                tc.tile_pool(name="qp", bufs=3) as qp,
                tc.tile_pool(name="ep", bufs=6) as ep,
                tc.tile_pool(name="op", bufs=2) as op,
                tc.tile_pool(name="rp", bufs=8) as rp,
                tc.tile_pool(name="sps", bufs=2, space="PSUM") as sps,
                tc.tile_pool(name="pvs", bufs=2, space="PSUM") as pvs,
            ):
                def emit_pv_norm(b, sT, eT, o_head, h):
                    # PV psum: col 512*pr + 130*j, tt = 2*pr + j
                    pvt = pvs.tile([128, 1024], F32, tag="pv", name="pvt")
                    for pr in range(2):
                        for j in range(2):
                            tt = 2 * pr + j
                            t = 4 * b + tt
                            K = 8 * t + 7
                            c0k = min(K, 128)
                            c1k = K - 128
                            out_ap = pvt[:, 512 * pr + 130 * j :
                                         512 * pr + 130 * j + 130]
                            nc.tensor.matmul(
                                out_ap,
                                eT[0:c0k, 128 * tt : 128 * (tt + 1)],
                                vca0[0:c0k, :],
                                start=True, stop=(c1k <= 0),
                            )
                            if c1k > 0:
                                nc.tensor.matmul(
                                    out_ap,
                                    eT[0:c1k, 512 + 128 * tt : 512 + 128 * (tt + 1)],
                                    vca1[0:c1k, :],
                                    start=False, stop=True,
                                )
                    # denominators at cols 128 + 512*pr + 130*j
                    pvt3 = pvt[:, :].rearrange("p (pr x) -> p pr x", pr=2)
                    den = pvt3[:, :, 128:259:130]  # [128, 2, 2]
                    rc = rp.tile([128, 4], F32, tag="rc", name="rc")
                    r4 = rc[:, :].rearrange("p (a b) -> p a b", a=2)
                    if b == 0:
                        rtmp = rp.tile([128, 4], F32, tag="rtmp", name="rt")
                        t4 = rtmp[:, :].rearrange("p (a b) -> p a b", a=2)
                        nc.vector.tensor_scalar_add(t4, den, 1e-30)
                        nc.vector.reciprocal(r4, t4)
                    else:
                        nc.vector.reciprocal(r4, den)
                    # one normalization mul: [128, (pr, j, vd)] * rc bcast
                    pv4 = pvt3[:, :, 0:260].rearrange(
                        "p pr (j x) -> p pr j x", j=2
                    )[:, :, :, 0:128]             # [128, 2, 2, 128]
                    rcb = rc[:, :].rearrange(
                        "p (a b) -> p a b", a=2
                    ).unsqueeze(3).broadcast_to([128, 2, 2, 128])
                    dst = o_head[:, 512 * b : 512 * (b + 1)].rearrange(
                        "p (pr j x) -> p pr j x", pr=2, j=2
                    )
                    nc.vector.tensor_mul(dst, pv4, rcb)
                    if b % 2 == 1:
                        qtr = (b - 1) // 2
                        nc.scalar.dma_start(
                            o_d.ap()[h].rearrange("p t v -> p (t v)")[
                                :, 1024 * qtr : 1024 * qtr + 1024],
                            o_head[:, 1024 * qtr : 1024 * qtr + 1024],
                        )

                def emit_qk_exp(b, qs, o_head, h, pv_prev=None):
                    mr = min(32 * b + 31, M)  # visible m count
                    c1r = mr - 128
                    nchunk = 1 if c1r <= 0 else 2
                    masked = [32 * (b - 4 * c) - 1 < 128 for c in range(nchunk)]
                    # QK psum: [128, 1024] = [chunk0 | chunk1]; both chunks
                    # compute all 128 rows (junk rows past the visible count
                    # are never read by PV)
                    sT = sps.tile([128, 1024], F32, tag="sT", name="sT")
                    for c in range(nchunk):
                        nc.tensor.matmul(
                            sT[:, 512 * c : 512 * c + 512],
                            kcT[:, 128 * c : 128 * c + 128],
                            qs,
                            start=True, stop=not masked[c],
                        )
                    if nchunk == 1 and pv_prev is not None:
                        # single-chunk blocks: one PV matmul of the previous
                        # block between QK and its mask hides the psum
                        # accumulation drain
                        emit_pv_norm(*pv_prev)
                        pv_prev = None
                    for c in range(nchunk):
                        if not masked[c]:
                            continue
                        v = b - 4 * c           # mask variant
                        # rows below the band add 0 (free: matmul cost is
                        # column-count only); base must be 32-aligned
                        be = min(128, 32 * v + 32)
                        mw = 32 if v == 4 else 512
                        nc.tensor.matmul(
                            sT[0:be, 512 * c : 512 * c + mw],
                            identb[:, 0:be],
                            maskv[:, 512 * v : 512 * v + mw],
                            start=False, stop=True,
                            skip_group_check=(mw != 512),
                        )
                    ecols = 512 * nchunk
                    eT = ep.tile([128, 1024], BF16, tag="eT", name="eT")
                    nc.scalar.activation(
                        eT[:, 0:ecols], sT[:, 0:ecols],
                        mybir.ActivationFunctionType.Exp, scale=SM,
                    )
                    if pv_prev is not None:
                        emit_pv_norm(*pv_prev)
                    return (b, sT, eT, o_head, h)

                # --- head 0: weave compression into the block pipeline ---
                o_head0 = op.tile([128, N], BF16, tag="o", name="oh")
                # K compression part A (chunks 0-16, psum borrowed from pvs)
                pk = pvs.tile([128, 1024], F32, tag="pv", name="pk")
                for c in range(17):
                    src = (ktA1[:, 128 * c : 128 * (c + 1)] if c < 9 else
                           ktA2[:, 128 * (c - 9) : 128 * (c - 8)])
                    nc.tensor.matmul(pk[:, 16 * c : 16 * c + 16], src,
                                     w01k[:, :], start=True, stop=True)
                pk3 = pk[:, 0:512].rearrange("p (t a) -> p t a", a=2)
                # kcT[d,m] = P0[m] + P1[m+1] + bias_k[d] (cols 0:128)
                nc.vector.tensor_scalar_add(kcT[:, 0:128], pk3[:, 0:128, 0],
                                            biask[:, 0:1])
                nc.vector.tensor_add(kcT[:, 0:128], kcT[:, 0:128],
                                     pk3[:, 1:129, 1])
                # QK/exp for blocks 0-3 (no PV yet: vca not ready)
                backlog = []
                for b in range(4):
                    backlog.append(
                        emit_qk_exp(b, q0a[:, 512 * b : 512 * b + 512],
                                    o_head0, 0))
                # K compression part B (chunks 17-31) -> kcT cols 128:255
                for c in range(17, 32):
                    src = ktB[:, 128 * (c - 17) : 128 * (c - 16)]
                    nc.tensor.matmul(pk[:, 16 * c : 16 * c + 16], src,
                                     w01k[:, :], start=True, stop=True)
                nc.vector.tensor_scalar_add(kcT[:, 128:M], pk3[:, 128:M, 0],
                                            biask[:, 0:1])
                nc.vector.tensor_add(kcT[:, 128:M], kcT[:, 128:M],
                                     pk3[:, 129 : M + 1, 1])
                nc.vector.memset(kcT[:, M:256], 0.0)
                # V compression
                pv = pvs.tile([128, 1024], F32, tag="pv", name="pvc")
                for c in range(32):
                    src = (vtA[:, 128 * c : 128 * (c + 1)] if c < 16 else
                           vtB[:, 128 * (c - 16) : 128 * (c - 15)])
                    nc.tensor.matmul(pv[:, 16 * c : 16 * c + 16], src,
                                     w01v[:, :], start=True, stop=True)
                pv3 = pv[:, 0:512].rearrange("p (t a) -> p t a", a=2)
                nc.vector.tensor_scalar_add(vcT[:, 0:M], pv3[:, 0:M, 0],
                                            biasv[:, 0:1])
                nc.vector.tensor_add(vcT[:, 0:M], vcT[:, 0:M],
                                     pv3[:, 1 : M + 1, 1])
                nc.vector.memset(vcT[:, M : M + 1], 0.0)
                tp = pvs.tile([128, 1024], F32, tag="pv", name="tp")
                nc.tensor.transpose(tp[:, 0:128], vcT[:, 0:128], ident[:, :])
                nc.tensor.transpose(tp[:, 128:256], vcT[:, 128:256],
                                    ident[:, :])
                nc.vector.tensor_copy(vca0[:, 0:128], tp[:, 0:128])
                nc.vector.tensor_copy(vca1[:, 0:128], tp[:, 128:256])
                # drain the PV backlog for blocks 0-2
                for pb in backlog[:3]:
                    emit_pv_norm(*pb)
                prev = backlog[3]
                # head 0 blocks 4-7 + heads 1-7, software-pipelined
                for h in range(HPC):
                    if h == 0:
                        qTh = None
                        o_head = o_head0
                        brange = range(4, NBLK)
                    else:
                        qTh = qp.tile([128, N], BF16, tag="qTh", name="qTh")
                        eng = nc.sync if h % 2 else nc.gpsimd
                        eng.dma_start(qTh[:, :], qT_d.ap()[h])
                        o_head = op.tile([128, N], BF16, tag="o", name="oh")
                        brange = range(NBLK)
                    for b in brange:
                        qs = (q0b[:, 512 * (b - 4) : 512 * (b - 3)]
                              if h == 0 else
                              qTh[:, 512 * b : 512 * (b + 1)])
                        prev = emit_qk_exp(b, qs, o_head, h, pv_prev=prev)
                emit_pv_norm(*prev)
    nc.compile()
    return nc


def make_consts(w_k, pe_k, w_v, pe_v):
    """Host-side constant tensors fed to every core."""
    f = np.float32
    w01k = np.zeros((128, 16), f)
    w01v = np.zeros((128, 16), f)
    for r in range(128):
        j = r // 16
        s = r % 16
        for a in range(2):
            # column layout (j, a): col = 2*j + a, matching psum (t, a)
            w01k[r, 2 * j + a] = w_k[16 * a + s]
            w01v[r, 2 * j + a] = w_v[16 * a + s]
    biask = (w_k[:, None] * pe_k).sum(0).astype(f)[:, None]  # [128,1]
    biasv = (w_v[:, None] * pe_v).sum(0).astype(f)[:, None]
    return {
        "w01k": np.ascontiguousarray(w01k).astype(ml_dtypes.bfloat16),
        "w01v": np.ascontiguousarray(w01v).astype(ml_dtypes.bfloat16),
        "biask": np.ascontiguousarray(biask),
        "biasv": np.ascontiguousarray(biasv),
        "ones1": np.hstack([np.ones((128, 1)), np.zeros((128, 1))]).astype(
            ml_dtypes.bfloat16),
    }


def make_in_map(q, k, v, consts, core):
    b, hq = core // 4, core % 4
    g = hq // 2
    qT = np.ascontiguousarray(
        q[b, :, 8 * hq : 8 * (hq + 1), :].transpose(1, 2, 0)
    ).astype(ml_dtypes.bfloat16)  # [8, D, N]
    # [N, D] -> SBUF tile layout [r=128, c=32, D] (r = row within chunk c)
    kk = np.ascontiguousarray(
        k[b, :, g, :].reshape(32, 128, D).transpose(1, 0, 2).reshape(128, 32 * D)
    ).astype(ml_dtypes.bfloat16)
    vv = np.ascontiguousarray(
        v[b, :, g, :].reshape(32, 128, D).transpose(1, 0, 2).reshape(128, 32 * D)
    ).astype(ml_dtypes.bfloat16)
    return {"qT": qT, "kk": kk, "vv": vv, **consts}


_CACHE = {}


def _compiled():
    if "nc" not in _CACHE:
        _CACHE["nc"] = build_program()
    return _CACHE["nc"]


def kernel(q, k, v, w_k, pe_k, w_v, pe_v, _trace=False, _trace_kwargs=None):
    q = np.asarray(q, np.float32)
    k = np.asarray(k, np.float32)
    v = np.asarray(v, np.float32)
    consts = make_consts(
        np.asarray(w_k, np.float32), np.asarray(pe_k, np.float32),
        np.asarray(w_v, np.float32), np.asarray(pe_v, np.float32),
    )
    nc = _compiled()
    in_maps = [make_in_map(q, k, v, consts, c) for c in range(8)]
    kw = {}
    if _trace:
        kw = {"trace": True, **(_trace_kwargs or {})}
    res = run_bass_kernel_spmd(nc, in_maps, core_ids=list(range(8)), **kw)
    out = np.empty((B, N, QH, VD), np.float32)
    for c in range(8):
        b, hq = c // 4, c % 4
        # o: [HPC, 128 p, 32 t, VD]; query n = 128*t + p
        oc = np.asarray(res.results[c]["o"], dtype=np.float32)
        out[b, :, 8 * hq : 8 * (hq + 1), :] = (
            oc.transpose(2, 1, 0, 3).reshape(N, HPC, VD)
        )
    _CACHE["last_result"] = res
    return out


# revision 42
# speedup vs baseline: 1.0070x; 1.0070x over previous
"""CompressAttn Trainium2 Bass kernel (final).

Problem: compressed-block attention.
  B=2, N=4096, QH=32, KH=2, D=VD=128, KSZ=32, STRIDE=16, M=255 blocks.
  kc[b,m,h,:] = sum_i w_k[i] * (k[b,16m+i,h,:] + pe_k[i,:])   (same for v)
  out = softmax(q @ kc^T * D^-0.5, causal-banded mask) @ vc, zero for n < 31.

Sharding: 8 cores = (batch b in {0,1}) x (query-head quarter hq in {0..3}).
Each core handles 8 query heads that share a single KV head (g = hq//2), so
K/V compression is done once per core.  No collectives needed; host gathers.

Device structure:
  - The causal staircase mask is ADDED into the QK psum by the tensor engine
    (selection-identity stationary x NEG-mask moving), so Scalar runs exactly
    one exp per (head, 512-query block) and Vector runs one reciprocal + one
    broadcast normalization multiply per block.
  - Mask variants and the identity matrices are generated on-device with
    affine_select (saves >1MB of startup DMA); k/v/q loads are split across
    the SP / Activation / Pool(SWDGE) DMA queues (~100GB/s each).
  - Head 0 is special-cased: K compression -> QK/exp of blocks 0-3 -> V
    compression (psum borrowed from the PV pool) -> PV backlog, so attention
    starts ~16us in instead of ~26us.
  - Per-block emission is software-pipelined: QK(b) chunks, then PV(b-1)
    and the mask matmuls ordered so psum accumulation drains stay hidden
    and the PE never stalls on exp; outputs are written bf16
    [128 part, 32 tile, vd] with four DMAs per head, host untangles.
"""

import ml_dtypes
import numpy as np

import concourse.bacc as bacc
import concourse.mybir as mybir
import concourse.tile as tile
from concourse.bass_utils import run_bass_kernel_spmd

# Problem geometry (hardcoded per contest rules).
B, N, QH, KH, D, VD = 2, 4096, 32, 2, 128, 128
KSZ, STRIDE = 32, 16
M = (N - KSZ) // STRIDE + 1          # 255 compressed blocks (m = 0..254)
HPC = QH // 4                         # 8 query heads per core
NBLK = N // 512                       # 8 query blocks of 512
SM = float(D) ** -0.5
NEGM = -16384.0                       # mask add; exp(SM*(-16384+s)) == 0

F32 = mybir.dt.float32
BF16 = mybir.dt.bfloat16

KTA_E = 17 * 128                      # ktile split: chunks 0-16 / 17-31
VTA_E = 16 * 128                      # vtile split: chunks 0-15 / 16-31


def build_program():
    nc = bacc.Bacc("TRN2", target_bir_lowering=False, debug=False)

    qT_d = nc.dram_tensor("qT", [HPC, D, N], BF16, kind="ExternalInput")
    # k/v pre-arranged on host to the SBUF tile layout [r, c, d] so loads
    # are one fully-contiguous descriptor per partition
    k_d = nc.dram_tensor("kk", [128, 32 * D], BF16, kind="ExternalInput")
    v_d = nc.dram_tensor("vv", [128, 32 * D], BF16, kind="ExternalInput")
    w01k_d = nc.dram_tensor("w01k", [128, 16], BF16, kind="ExternalInput")
    w01v_d = nc.dram_tensor("w01v", [128, 16], BF16, kind="ExternalInput")
    bk_d = nc.dram_tensor("biask", [128, 1], F32, kind="ExternalInput")
    bv_d = nc.dram_tensor("biasv", [128, 1], F32, kind="ExternalInput")
    ones_d = nc.dram_tensor("ones1", [128, 2], BF16, kind="ExternalInput")
    o_d = nc.dram_tensor("o", [HPC, 128, N // 128, VD], BF16,
                         kind="ExternalOutput")

    with tile.TileContext(nc) as tc:
        with tc.tile_pool(name="consts", bufs=1) as cp:
            w01k = cp.tile([128, 16], BF16)
            w01v = cp.tile([128, 16], BF16)
            biask = cp.tile([128, 1], F32)
            biasv = cp.tile([128, 1], F32)
            maskv = cp.tile([128, 8 * 512], BF16)
            mvf = cp.tile([128, 512], F32)
            zf = cp.tile([128, 512], F32)
            ident = cp.tile([128, 128], F32)
            identb = cp.tile([128, 128], BF16)
            tmpb = cp.tile([128, 128], BF16)
            tmpb2 = cp.tile([128, 128], BF16)
            tmpf = cp.tile([128, 128], F32)
            tmpf2 = cp.tile([128, 128], F32)
            ktA1 = cp.tile([128, 9 * 128], BF16)
            ktA2 = cp.tile([128, 8 * 128], BF16)
            ktB = cp.tile([128, 32 * 128 - KTA_E], BF16)
            vtA = cp.tile([128, VTA_E], BF16)
            vtB = cp.tile([128, 32 * 128 - VTA_E], BF16)
            kcT = cp.tile([128, 256], BF16)       # [d, m] (col 255 zero pad)
            vcT = cp.tile([128, 256], F32)        # [d, t] staging
            vca0 = cp.tile([128, 130], BF16)      # [m 0:128,   vc|1|0]
            vca1 = cp.tile([128, 130], BF16)      # [m 128:255, vc|1|0]
            q0a = cp.tile([128, 2048], BF16)      # head-0 q, blocks 0-3
            q0b = cp.tile([128, 2048], BF16)      # head-0 q, blocks 4-7

            # --- DMA schedule: 3 queues in parallel ---
            nc.sync.dma_start(w01k[:, :], w01k_d.ap())
            nc.sync.dma_start(w01v[:, :], w01v_d.ap())
            nc.sync.dma_start(ktA1[:, :], k_d.ap()[:, 0 : 9 * 128])
            nc.sync.dma_start(q0a[:, :], qT_d.ap()[0, :, 0:2048])
            nc.sync.dma_start(ktB[:, :], k_d.ap()[:, KTA_E:])
            nc.scalar.dma_start(biask[:, :], bk_d.ap())
            nc.scalar.dma_start(biasv[:, :], bv_d.ap())
            nc.scalar.dma_start(ktA2[:, :], k_d.ap()[:, 9 * 128 : KTA_E])
            nc.scalar.dma_start(q0b[:, :], qT_d.ap()[0, :, 2048:4096])
            nc.gpsimd.dma_start(vtA[:, :], v_d.ap()[:, 0:VTA_E])
            nc.gpsimd.dma_start(vtB[:, :], v_d.ap()[:, VTA_E:])
            nc.gpsimd.dma_start(vca0[:, 128:130], ones_d.ap())
            nc.gpsimd.dma_start(vca1[:, 128:130], ones_d.ap())

            # --- on-device constants (Pool affine_select + DVE copies,
            # overlapping the DMA waits); identities first (every mask
            # matmul needs identb), then mask variants in usage order
            nc.vector.memset(tmpb[:, :], 1.0)
            nc.gpsimd.affine_select(
                tmpb2[:, :], tmpb[:, :], pattern=[[1, 128]],
                compare_op=mybir.AluOpType.is_ge, fill=0.0,
                base=0, channel_multiplier=-1,
            )
            nc.gpsimd.affine_select(
                identb[:, :], tmpb2[:, :], pattern=[[-1, 128]],
                compare_op=mybir.AluOpType.is_ge, fill=0.0,
                base=0, channel_multiplier=1,
            )
            nc.vector.memset(tmpf[:, :], 1.0)
            nc.gpsimd.affine_select(
                tmpf2[:, :], tmpf[:, :], pattern=[[1, 128]],
                compare_op=mybir.AluOpType.is_ge, fill=0.0,
                base=0, channel_multiplier=-1,
            )
            nc.gpsimd.affine_select(
                ident[:, :], tmpf2[:, :], pattern=[[-1, 128]],
                compare_op=mybir.AluOpType.is_ge, fill=0.0,
                base=0, channel_multiplier=1,
            )
            # mask variant v, row p, col j: NEGM iff j < 16*p + 31 - 512*v
            nc.vector.memset(zf[:, :], 0.0)
            for v in range(8):
                nc.gpsimd.affine_select(
                    mvf[:, :], zf[:, :], pattern=[[1, 512]],
                    compare_op=mybir.AluOpType.is_ge, fill=NEGM,
                    base=512 * v - 31, channel_multiplier=-16,
                )
                nc.vector.tensor_copy(maskv[:, 512 * v : 512 * v + 512],
                                      mvf[:, :])

            # ---- attention (+ compression woven into head 0) ----
            with (
                tc.tile_pool(name="qp", bufs=3) as qp,
                tc.tile_pool(name="ep", bufs=6) as ep,
                tc.tile_pool(name="op", bufs=2) as op,
                tc.tile_pool(name="rp", bufs=8) as rp,
                tc.tile_pool(name="sps", bufs=2, space="PSUM") as sps,
                tc.tile_pool(name="pvs", bufs=2, space="PSUM") as pvs,
            ):
                def emit_pv_norm(b, sT, eT, o_head, h):
                    # PV psum: col 512*pr + 130*j, tt = 2*pr + j
                    pvt = pvs.tile([128, 1024], F32, tag="pv", name="pvt")
                    for pr in range(2):
                        for j in range(2):
                            tt = 2 * pr + j
                            t = 4 * b + tt
                            K = 8 * t + 7
                            c0k = min(K, 128)
                            c1k = K - 128
                            out_ap = pvt[:, 512 * pr + 130 * j :
                                         512 * pr + 130 * j + 130]
                            nc.tensor.matmul(
                                out_ap,
                                eT[0:c0k, 128 * tt : 128 * (tt + 1)],
                                vca0[0:c0k, :],
                                start=True, stop=(c1k <= 0),
                            )
                            if c1k > 0:
                                nc.tensor.matmul(
                                    out_ap,
                                    eT[0:c1k, 512 + 128 * tt : 512 + 128 * (tt + 1)],
                                    vca1[0:c1k, :],
                                    start=False, stop=True,
                                )
                    # denominators at cols 128 + 512*pr + 130*j
                    pvt3 = pvt[:, :].rearrange("p (pr x) -> p pr x", pr=2)
                    den = pvt3[:, :, 128:259:130]  # [128, 2, 2]
                    rc = rp.tile([128, 4], F32, tag="rc", name="rc")
                    r4 = rc[:, :].rearrange("p (a b) -> p a b", a=2)
                    if b == 0:
                        rtmp = rp.tile([128, 4], F32, tag="rtmp", name="rt")
                        t4 = rtmp[:, :].rearrange("p (a b) -> p a b", a=2)
                        nc.vector.tensor_scalar_add(t4, den, 1e-30)
                        nc.vector.reciprocal(r4, t4)
                    else:
                        nc.vector.reciprocal(r4, den)
                    # one normalization mul: [128, (pr, j, vd)] * rc bcast
                    pv4 = pvt3[:, :, 0:260].rearrange(
                        "p pr (j x) -> p pr j x", j=2
                    )[:, :, :, 0:128]             # [128, 2, 2, 128]
                    rcb = rc[:, :].rearrange(
                        "p (a b) -> p a b", a=2
                    ).unsqueeze(3).broadcast_to([128, 2, 2, 128])
                    dst = o_head[:, 512 * b : 512 * (b + 1)].rearrange(
                        "p (pr j x) -> p pr j x", pr=2, j=2
                    )
                    nc.vector.tensor_mul(dst, pv4, rcb)
                    if b % 2 == 1:
                        qtr = (b - 1) // 2
                        nc.scalar.dma_start(
                            o_d.ap()[h].rearrange("p t v -> p (t v)")[
                                :, 1024 * qtr : 1024 * qtr + 1024],
                            o_head[:, 1024 * qtr : 1024 * qtr + 1024],
                        )

                def emit_qk_exp(b, qs, o_head, h, pv_prev=None):
                    mr = min(32 * b + 31, M)  # visible m count
                    c1r = mr - 128
                    nchunk = 1 if c1r <= 0 else 2
                    masked = [32 * (b - 4 * c) - 1 < 128 for c in range(nchunk)]
                    # QK psum: [128, 1024] = [chunk0 | chunk1]; both chunks
                    # compute all 128 rows (junk rows past the visible count
                    # are never read by PV)
                    sT = sps.tile([128, 1024], F32, tag="sT", name="sT")
                    for c in range(nchunk):
                        nc.tensor.matmul(
                            sT[:, 512 * c : 512 * c + 512],
                            kcT[:, 128 * c : 128 * c + 128],
                            qs,
                            start=True, stop=not masked[c],
                        )
                    if nchunk == 1 and pv_prev is not None:
                        # single-chunk blocks: one PV matmul of the previous
                        # block between QK and its mask hides the psum
                        # accumulation drain
                        emit_pv_norm(*pv_prev)
                        pv_prev = None
                    for c in range(nchunk):
                        if not masked[c]:
                            continue
                        v = b - 4 * c           # mask variant
                        # rows below the band add 0 (free: matmul cost is
                        # column-count only); base must be 32-aligned
                        be = min(128, 32 * v + 32)
                        mw = 32 if v == 4 else 512
                        nc.tensor.matmul(
                            sT[0:be, 512 * c : 512 * c + mw],
                            identb[:, 0:be],
                            maskv[:, 512 * v : 512 * v + mw],
                            start=False, stop=True,
                            skip_group_check=(mw != 512),
                        )
                    ecols = 512 * nchunk
                    eT = ep.tile([128, 1024], BF16, tag="eT", name="eT")
                    nc.scalar.activation(
                        eT[:, 0:ecols], sT[:, 0:ecols],
                        mybir.ActivationFunctionType.Exp, scale=SM,
                    )
                    if pv_prev is not None:
                        emit_pv_norm(*pv_prev)
                    return (b, sT, eT, o_head, h)

                # --- head 0: weave compression into the block pipeline ---
                o_head0 = op.tile([128, N], BF16, tag="o", name="oh")
                # K compression part A (chunks 0-16, psum borrowed from pvs)
                pk = pvs.tile([128, 1024], F32, tag="pv", name="pk")
                for c in range(17):
                    src = (ktA1[:, 128 * c : 128 * (c + 1)] if c < 9 else
                           ktA2[:, 128 * (c - 9) : 128 * (c - 8)])
                    nc.tensor.matmul(pk[:, 16 * c : 16 * c + 16], src,
                                     w01k[:, :], start=True, stop=True)
                pk3 = pk[:, 0:512].rearrange("p (t a) -> p t a", a=2)
                # kcT[d,m] = P0[m] + P1[m+1] + bias_k[d] (cols 0:128)
                nc.vector.tensor_scalar_add(kcT[:, 0:128], pk3[:, 0:128, 0],
                                            biask[:, 0:1])
                nc.vector.tensor_add(kcT[:, 0:128], kcT[:, 0:128],
                                     pk3[:, 1:129, 1])
                # QK/exp for blocks 0-3 (no PV yet: vca not ready)
                backlog = []
                for b in range(4):
                    backlog.append(
                        emit_qk_exp(b, q0a[:, 512 * b : 512 * b + 512],
                                    o_head0, 0))
                # K compression part B (chunks 17-31) -> kcT cols 128:255
                for c in range(17, 32):
                    src = ktB[:, 128 * (c - 17) : 128 * (c - 16)]
                    nc.tensor.matmul(pk[:, 16 * c : 16 * c + 16], src,
                                     w01k[:, :], start=True, stop=True)
                nc.vector.tensor_scalar_add(kcT[:, 128:M], pk3[:, 128:M, 0],
                                            biask[:, 0:1])
                nc.vector.tensor_add(kcT[:, 128:M], kcT[:, 128:M],
                                     pk3[:, 129 : M + 1, 1])
                nc.vector.memset(kcT[:, M:256], 0.0)
                # V compression
                pv = pvs.tile([128, 1024], F32, tag="pv", name="pvc")
                for c in range(32):
                    src = (vtA[:, 128 * c : 128 * (c + 1)] if c < 16 else
                           vtB[:, 128 * (c - 16) : 128 * (c - 15)])
                    nc.tensor.matmul(pv[:, 16 * c : 16 * c + 16], src,
                                     w01v[:, :], start=True, stop=True)
                pv3 = pv[:, 0:512].rearrange("p (t a) -> p t a", a=2)
                nc.vector.tensor_scalar_add(vcT[:, 0:M], pv3[:, 0:M, 0],
                                            biasv[:, 0:1])
                nc.vector.tensor_add(vcT[:, 0:M], vcT[:, 0:M],
                                     pv3[:, 1 : M + 1, 1])
                nc.vector.memset(vcT[:, M : M + 1], 0.0)
                tp = pvs.tile([128, 1024], F32, tag="pv", name="tp")
                nc.tensor.transpose(tp[:, 0:128], vcT[:, 0:128], ident[:, :])
                nc.tensor.transpose(tp[:, 128:256], vcT[:, 128:256],
                                    ident[:, :])
                nc.vector.tensor_copy(vca0[:, 0:128], tp[:, 0:128])
                nc.vector.tensor_copy(vca1[:, 0:128], tp[:, 128:256])
                # drain the PV backlog for blocks 0-2
                for pb in backlog[:3]:
                    emit_pv_norm(*pb)
                prev = backlog[3]
                # head 0 blocks 4-7 + heads 1-7, software-pipelined
                for h in range(HPC):
                    if h == 0:
                        qTh = None
                        o_head = o_head0
                        brange = range(4, NBLK)
                    else:
                        qTh = qp.tile([128, N], BF16, tag="qTh", name="qTh")
                        eng = nc.sync if h % 2 else nc.gpsimd
                        eng.dma_start(qTh[:, :], qT_d.ap()[h])
                        o_head = op.tile([128, N], BF16, tag="o", name="oh")
                        brange = range(NBLK)
                    for b in brange:
                        qs = (q0b[:, 512 * (b - 4) : 512 * (b - 3)]
                              if h == 0 else
                              qTh[:, 512 * b : 512 * (b + 1)])
                        prev = emit_qk_exp(b, qs, o_head, h, pv_prev=prev)
                emit_pv_norm(*prev)
    nc.compile()
    return nc


def make_consts(w_k, pe_k, w_v, pe_v):
    """Host-side constant tensors fed to every core."""
    f = np.float32
    w01k = np.zeros((128, 16), f)
    w01v = np.zeros((128, 16), f)
    for r in range(128):
        j = r // 16
        s = r % 16
        for a in range(2):
            # column layout (j, a): col = 2*j + a, matching psum (t, a)
            w01k[r, 2 * j + a] = w_k[16 * a + s]
            w01v[r, 2 * j + a] = w_v[16 * a + s]
    biask = (w_k[:, None] * pe_k).sum(0).astype(f)[:, None]  # [128,1]
    biasv = (w_v[:, None] * pe_v).sum(0).astype(f)[:, None]
    return {
        "w01k": np.ascontiguousarray(w01k).astype(ml_dtypes.bfloat16),
        "w01v": np.ascontiguousarray(w01v).astype(ml_dtypes.bfloat16),
        "biask": np.ascontiguousarray(biask),
        "biasv": np.ascontiguousarray(biasv),
        "ones1": np.hstack([np.ones((128, 1)), np.zeros((128, 1))]).astype(
            ml_dtypes.bfloat16),
    }


def make_in_map(q, k, v, consts, core):
    b, hq = core // 4, core % 4
    g = hq // 2
    qT = np.ascontiguousarray(
        q[b, :, 8 * hq : 8 * (hq + 1), :].transpose(1, 2, 0)
    ).astype(ml_dtypes.bfloat16)  # [8, D, N]
    # [N, D] -> SBUF tile layout [r=128, c=32, D] (r = row within chunk c)
    kk = np.ascontiguousarray(
        k[b, :, g, :].reshape(32, 128, D).transpose(1, 0, 2).reshape(128, 32 * D)
    ).astype(ml_dtypes.bfloat16)
    vv = np.ascontiguousarray(
        v[b, :, g, :].reshape(32, 128, D).transpose(1, 0, 2).reshape(128, 32 * D)
    ).astype(ml_dtypes.bfloat16)
    return {"qT": qT, "kk": kk, "vv": vv, **consts}


_CACHE = {}


def _compiled():
    if "nc" not in _CACHE:
        _CACHE["nc"] = build_program()
    return _CACHE["nc"]


def kernel(q, k, v, w_k, pe_k, w_v, pe_v, _trace=False, _trace_kwargs=None):
    q = np.asarray(q, np.float32)
    k = np.asarray(k, np.float32)
    v = np.asarray(v, np.float32)
    consts = make_consts(
        np.asarray(w_k, np.float32), np.asarray(pe_k, np.float32),
        np.asarray(w_v, np.float32), np.asarray(pe_v, np.float32),
    )
    nc = _compiled()
    in_maps = [make_in_map(q, k, v, consts, c) for c in range(8)]
    kw = {}
    if _trace:
        kw = {"trace": True, **(_trace_kwargs or {})}
    res = run_bass_kernel_spmd(nc, in_maps, core_ids=list(range(8)), **kw)
    out = np.empty((B, N, QH, VD), np.float32)
    for c in range(8):
        b, hq = c // 4, c % 4
        # o: [HPC, 128 p, 32 t, VD]; query n = 128*t + p
        oc = np.asarray(res.results[c]["o"], dtype=np.float32)
        out[b, :, 8 * hq : 8 * (hq + 1), :] = (
            oc.transpose(2, 1, 0, 3).reshape(N, HPC, VD)
        )
    _CACHE["last_result"] = res
    return out
